# revision 5
# baseline (speedup 1.0000x reference)
"""Trainium2 Bass kernel for nn_MixDimensionEmbeddingBag (B=16384, F=26, D=64).

v2 strategy (4-queue SWDGE dma_gather pipeline; ~2x the v1 indirect-DMA
issue rate, which was the measured bottleneck at ~641us/core):

  Host: fold the projectors into the tables (t1p = t1@W1.T, t2p = t2@W2.T)
  so every embedding row is 64 f32 = 256B (the dma_gather minimum).  Merge
  [t0; t1p; t2p] into one 2.6M-row space.  Per core (2048 samples), sort
  each half's 1024x26 lookups by row id and bucket into 80 windows of
  32767 rows (int16-indexable).  Bias enters as one PE rank-1 matmul.

  Device, per core:
   leg1: per (half, window): one dma_gather (int16 window-local indices,
     compile-time capacity = roundup(max-over-cores count, 128), pad
     indices point at the window base row) on SWDGE queues 0-3 -- the 4
     queue contexts generate descriptors on parallel Q7 cores (measured
     4x vs 1 queue) -- then one HWDGE direct write of the gathered tile
     to a DRAM staging buffer at the window's baked offset (sorted
     order).  WAW edges between the disjoint staging writes are stripped.
   leg2: staging halves are < 32767 rows, so a second round of
     dma_gathers pulls rows back sample-grouped: for each 128-sample
     group, 4 gathers fetch its 26 rows as slots (idx = baked-layout
     sortpos, shipped per-core at runtime; pad slots hit a never-written
     zero row).  DVE tensor_reduce sums the slots, bias PSUM is added,
     and the [128,64] group result DMAs straight to the output rows.

  Scatter-add was rejected: CCE add loses colliding updates on this HW
  (measured), so routing uses a second gather pass instead.
"""

import numpy as np

B = 16384
F = 26
FIELD_DIM = 100000
D = 64
N_CORES = 8
P = 128
BPC = 2048            # samples per core
HALF = 1024           # samples per half
NROWS = F * FIELD_DIM # merged row space (2.6M)
WIN = 32767           # rows per leg1 window (int16 addressable)
NW = -(-NROWS // WIN) # 80 windows
SLOTS = F             # rows per sample
_nc_cache = {}


def _build(caps, stg_rows, loop_k=None):
    """caps: [2][NW] int, each a multiple of 128 (<=1024). stg_rows: rows
    per staging half (zero row at stg_rows-1)."""
    import sys
    try:
        from concourse import bass, bacc, mybir, tile, library_config
    except ImportError:
        sys.path.insert(0, "/opt/trn_rl_repo")
        from concourse import bass, bacc, mybir, tile, library_config

    f32 = mybir.dt.float32
    i16 = mybir.dt.int16
    nc = bacc.Bacc("TRN2", target_bir_lowering=False, debug=False,
                   num_swdge_queues=4)

    gcols = sum(c // 16 for h in range(2) for c in caps[h])
    rcols = 16 * (3 * 64 + 16)   # 16 groups x (3x1024 + 256) idx / 16

    R = nc.dram_tensor("R", [NROWS, D], f32, kind="ExternalInput")
    gi = nc.dram_tensor("gi", [P, gcols], i16, kind="ExternalInput")
    ri = nc.dram_tensor("ri", [P, rcols], i16, kind="ExternalInput")
    onesd = nc.dram_tensor("ones", [1, P], f32, kind="ExternalInput")
    bvd = nc.dram_tensor("bv", [1, D], f32, kind="ExternalInput")
    stg = nc.dram_tensor("stg", [2, stg_rows, D], f32, kind="ExternalOutput")
    out = nc.dram_tensor("out", [BPC, D], f32, kind="ExternalOutput")

    AX = mybir.AxisListType.X
    add = mybir.AluOpType.add

    with tile.TileContext(nc) as tc:
        nc.gpsimd.load_library(library_config.mlp)
        with tc.tile_pool(name="const", bufs=1) as cpool, \
             tc.tile_pool(name="g1", bufs=24) as gpool, \
             tc.tile_pool(name="g2", bufs=16) as rpool, \
             tc.tile_pool(name="pa", bufs=12) as ppool, \
             tc.tile_pool(name="fin", bufs=4) as fpool, \
             tc.tile_pool(name="ps", bufs=1, space="PSUM") as pspool:
            gi_sb = cpool.tile([P, gcols], i16)
            ri_sb = cpool.tile([P, rcols], i16)
            ones_sb = cpool.tile([1, P], f32)
            bv_sb = cpool.tile([1, D], f32)
            nc.sync.dma_start(out=gi_sb[:], in_=gi[:])
            nc.sync.dma_start(out=ri_sb[:], in_=ri[:])
            nc.sync.dma_start(out=ones_sb[:], in_=onesd[:])
            nc.sync.dma_start(out=bv_sb[:], in_=bvd[:])

            psb = pspool.tile([P, D], f32)
            nc.tensor.matmul(psb[:], lhsT=ones_sb[:], rhs=bv_sb[:],
                             start=True, stop=True)

            def body():
                q = 0
                gcol = 0
                write_names = set()
                hwdge = [nc.sync, nc.scalar]
                for h in range(2):
                    off = 0
                    for w in range(NW):
                        C = caps[h][w]          # 16-aligned gather count
                        SL = -(-C // 128)       # staging slots (128 rows)
                        w0 = w * WIN
                        w1 = min(w0 + WIN, NROWS)
                        gt = gpool.tile([P, SL, D], f32)
                        nc.gpsimd.dma_gather(
                            out_ap=gt[:], in_ap=R[w0:w1, :],
                            idxs_ap=gi_sb[:, gcol:gcol + C // 16],
                            num_idxs=C, num_idxs_reg=C, elem_size=D,
                            queue_num=q % 4)
                        q += 1
                        gcol += C // 16
                        wr = hwdge[w % 2].dma_start(
                            out=stg[h, off:off + SL * P, :].rearrange(
                                "(sl p) d -> p sl d", p=P),
                            in_=gt[:])
                        mi = wr.ins
                        for dname in list(mi.sync_dependency_names()):
                            if dname in write_names:
                                mi.remove_dependency(dname)
                        write_names.add(mi.name)
                        off += SL * P

                rcol = 0
                gidx = 0
                for h in range(2):
                    stg_h = stg[h, :, :]
                    for s7 in range(8):
                        parts = []
                        for jb in range(4):
                            n = 1024 if jb < 3 else 256
                            sl = n // 128
                            rt = rpool.tile([P, sl, D], f32)
                            nc.gpsimd.dma_gather(
                                out_ap=rt[:], in_ap=stg_h,
                                idxs_ap=ri_sb[:, rcol:rcol + n // 16],
                                num_idxs=n, num_idxs_reg=n, elem_size=D,
                                queue_num=q % 4)
                            q += 1
                            rcol += n // 16
                            pt = ppool.tile([P, D], f32)
                            nc.vector.tensor_reduce(
                                out=pt[:],
                                in_=rt[:].rearrange("p sl d -> p d sl"),
                                axis=AX, op=add)
                            parts.append(pt)
                        s01 = ppool.tile([P, D], f32)
                        s23 = ppool.tile([P, D], f32)
                        fin = fpool.tile([P, D], f32)
                        nc.vector.tensor_add(out=s01[:], in0=parts[0][:],
                                             in1=parts[1][:])
                        nc.vector.tensor_add(out=s23[:], in0=parts[2][:],
                                             in1=parts[3][:])
                        nc.vector.tensor_add(out=s01[:], in0=s01[:],
                                             in1=s23[:])
                        nc.vector.tensor_add(out=fin[:], in0=s01[:],
                                             in1=psb[:])
                        hwdge[gidx % 2].dma_start(
                            out=out[h * HALF + s7 * P:
                                    h * HALF + s7 * P + P, :],
                            in_=fin[:])
                        gidx += 1

            if loop_k:
                with tc.For_i(0, loop_k, 1):
                    body()
            else:
                body()

    nc.compile()
    return nc


def _prep(x, t0, t1, t2, W1, b1, W2, b2):
    x = np.asarray(x).astype(np.int64, copy=False)
    t0 = np.asarray(t0, dtype=np.float32)
    t1 = np.asarray(t1, dtype=np.float32)
    t2 = np.asarray(t2, dtype=np.float32)
    W1 = np.asarray(W1, dtype=np.float32)
    W2 = np.asarray(W2, dtype=np.float32)
    b1 = np.asarray(b1, dtype=np.float32)
    b2 = np.asarray(b2, dtype=np.float32)

    t1p = t1 @ W1.T            # [8*FIELD_DIM, 64]
    t2p = t2 @ W2.T            # [10*FIELD_DIM, 64]
    R = np.ascontiguousarray(np.concatenate([t0, t1p, t2p], axis=0))
    bv = (8.0 * b1 + 10.0 * b2).astype(np.float32).reshape(1, D)

    fbase = (np.arange(F, dtype=np.int64) * FIELD_DIM)[None, :]

    # per (core, half, window) counts -> baked caps
    counts = np.zeros((N_CORES, 2, NW), np.int64)
    per_core = []
    for c in range(N_CORES):
        xs = x[c * BPC:(c + 1) * BPC]
        g = xs + fbase                        # [2048, 26] global rows
        halves = []
        for h in range(2):
            gh = g[h * HALF:(h + 1) * HALF]   # [1024, 26]
            e_g = gh.ravel()
            e_s = np.repeat(np.arange(HALF), F)
            order = np.argsort(e_g, kind="stable")
            sg = e_g[order]
            ss = e_s[order]
            w_arr = sg // WIN
            counts[c, h] = np.bincount(w_arr, minlength=NW)
            halves.append((sg, ss, w_arr))
        per_core.append(halves)

    caps = [[0] * NW for _ in range(2)]
    for h in range(2):
        for w in range(NW):
            m = int(counts[:, h, w].max())
            caps[h][w] = max(16, -(-m // 16) * 16)
            assert caps[h][w] <= 1024, (h, w, m)
    strides = [[-(-c // 128) * 128 for c in caps[h]] for h in range(2)]
    stg_h_sizes = [sum(strides[h]) for h in range(2)]
    stg_rows = max(stg_h_sizes) + 128          # zero row at stg_rows-1
    assert stg_rows - 1 <= 32767, stg_rows
    zrow = stg_rows - 1
    offs = [np.concatenate([[0], np.cumsum(strides[h])[:-1]])
            for h in range(2)]

    def wrap(flat):
        n = flat.shape[0]
        return np.tile(np.ascontiguousarray(
            flat.reshape(n // 16, 16).T), (8, 1))

    in_maps = []
    for c in range(N_CORES):
        gi_parts = []
        ri_parts = []
        for h in range(2):
            sg, ss, w_arr = per_core[c][h]
            n_h = sg.shape[0]
            # leg1 gather idx per window: window-local rows, padded with 0
            loc = (sg - w_arr * WIN).astype(np.int16)
            wstart = np.concatenate(
                [[0], np.cumsum(counts[c, h])[:-1]]).astype(np.int64)
            # staging position of sorted entry j
            rank = np.arange(n_h) - wstart[w_arr]
            stgpos = (offs[h][w_arr] + rank).astype(np.int64)
            for w in range(NW):
                C = caps[h][w]
                lw = np.zeros(C, np.int16)
                cnt = int(counts[c, h, w])
                lw[:cnt] = loc[wstart[w]:wstart[w] + cnt]
                gi_parts.append(wrap(lw))
            # slot index k per (sample): rank among the sample's sorted rows
            by_s = np.lexsort((np.arange(n_h), ss))
            k_sorted = np.arange(n_h) - np.repeat(
                np.arange(HALF) * F, F)        # ranks 0..25 within group
            k_arr = np.empty(n_h, np.int64)
            k_arr[by_s] = k_sorted
            # tok_pos[s, k] = staging row of that entry
            tok_pos = np.empty((HALF, F), np.int64)
            tok_pos[ss, k_arr] = stgpos
            # leg2 idx: group (s7), 4 instrs: slots 0-7, 8-15, 16-23, 24-25
            for s7 in range(8):
                samp = s7 * P + np.arange(P)   # [128]
                for jb in range(4):
                    nsl = 8 if jb < 3 else 2
                    sl = jb * 8 + np.arange(nsl)
                    blk = np.full((nsl, P), zrow, np.int64)
                    blk[:, :] = tok_pos[samp][:, sl].T
                    ri_parts.append(wrap(blk.ravel().astype(np.int16)))
        in_maps.append({
            "R": R,
            "gi": np.concatenate(gi_parts, axis=1),
            "ri": np.concatenate(ri_parts, axis=1),
            "ones": np.ones((1, P), np.float32),
            "bv": bv,
        })
    return in_maps, caps, stg_rows


def kernel(x, t0, t1, t2, W1, b1, W2, b2, _trace=False, _loop_k=None):
    global _nc_cache
    import sys
    try:
        from concourse.bass_utils import run_bass_kernel_spmd
    except ImportError:
        sys.path.insert(0, "/opt/trn_rl_repo")
        from concourse.bass_utils import run_bass_kernel_spmd

    in_maps, caps, stg_rows = _prep(x, t0, t1, t2, W1, b1, W2, b2)
    key = (tuple(caps[0]), tuple(caps[1]), stg_rows, _loop_k)
    if key not in _nc_cache:
        _nc_cache[key] = _build(caps, stg_rows, loop_k=_loop_k)
    res = run_bass_kernel_spmd(
        _nc_cache[key], in_maps, list(range(N_CORES)), trace=_trace)
    outp = np.concatenate(
        [np.asarray(res.results[c]["out"]) for c in range(N_CORES)], axis=0)
    if _trace:
        return outp, res
    return outp
